# revision 6
# baseline (speedup 1.0000x reference)
"""ContrastiveLoss kernel for 8x Trainium2 NeuronCores.

Math: loss = mean_i ||o2_i - o1_i||^2  +  mean_i relu(MARGIN - d_i)
where d_i is the rn_i-th smallest entry of row i of the [N,N] distance
matrix dist(i,j) = ||o2_j - o1_i||.

Every candidate d_i is >= min_j dist(i,j), so whenever we can PROVE
min_j dist(i,j) >= MARGIN for all i the margin term is exactly 0 and
loss == mean(pos), which the host computes exactly in fp64.

The proof splits the feature dim into a 512-dim head and 512-dim tail:
    <a_i, b_j> <= <a_head_i, b_head_j> + |a_tail_i| * max_j |b_tail_j|
The head inner products are bounded on-device: each core computes its
fp8 DoubleRow Gram block (the dominant compute) and reduces each
[128, 2048] PSUM tile to either an exact row max (DVE) or a
log-sum-exp upper bound (ACT, sum_j exp(T*g)); the host combines them
with rigorous fp8-rounding slack. The tail norms, full row norms, and
positive loss are tiny O(N*D) host work in fp64. For the reference's
gaussian inputs the certified bound clears the threshold by >200 (see
LAST_BOUND); if the check ever failed, an exact host fallback
reproduces the reference computation.

Sharding: 4 row-blocks x 2 col-halves. Core c owns rows
(c//2)*2048 .. +2048 of output1 and cols (c%2)*4096 .. +4096 of
output2: per-core DMA is 1 MB lhsT + 2 MB rhsT (fp8 head only),
split across the gpsimd SWDGE queue and both HWDGE queues (SP, ACT).
"""

import numpy as np
import ml_dtypes

N = 8192
D = 1024
HK = 512            # head dims bounded on-device; D-HK tail bounded on host
NCORES = 8
RB = 4              # row blocks
CBn = 2             # col halves
MR = N // RB        # 2048 rows per core
MC = N // CBn       # 4096 cols per core
P = 128
KT = HK // P        # 4 k-tiles
KP = KT // 2        # 2 DoubleRow k-pairs
MT = MR // P        # 16 m-tiles per core
NFREE = 512         # matmul free dim
CW = 2048           # psum tile width (4 banks)
NCG = MC // CW      # 2 column groups per core
GC = MT * NCG       # 32 reduce columns per core
MARGIN = 2.0
QUANT = 30
T_LSE = 0.25        # lse temperature: ln(sexp)/T + 1 >= block row max

_PROGS = {}
LAST_RESULTS = None
LAST_BOUND = None
LAST_FASTPATH = None


def _build_program(reps=1, mode="full"):
    """mode: 'full' (normal), 'dma' (loads only), 'mm' (loads hoisted,
    matmuls + token drains), 'compute' (loads hoisted, real drains)."""
    import contextlib

    import concourse.bacc as bacc
    import concourse.mybir as mybir
    import concourse.tile as tile

    nc = bacc.Bacc(None, target_bir_lowering=False, debug=False)
    f32 = mybir.dt.float32
    fp8 = mybir.dt.float8e4
    X = mybir.AxisListType.X
    Alu = mybir.AluOpType
    DR = mybir.MatmulPerfMode.DoubleRow

    lhsT_d = nc.dram_tensor("lhsT", [HK, MR], fp8, kind="ExternalInput")
    rhsT_d = nc.dram_tensor("rhsT", [HK, MC], fp8, kind="ExternalInput")
    # col = m*NCG + cg; (m+cg) even -> DVE exact max in gmax, odd -> ACT
    # sum_j exp(T_LSE*g) in sexp. Unwritten cols of the other tensor stay 0.
    gmax_d = nc.dram_tensor("gmax", [P, GC], f32, kind="ExternalOutput")
    sexp_d = nc.dram_tensor("sexp", [P, GC], f32, kind="ExternalOutput")

    with tile.TileContext(nc) as tc:
        with (
            tc.tile_pool(name="persist", bufs=1) as persist,
            tc.tile_pool(name="lpool", bufs=2) as lpool,
            tc.tile_pool(name="rpool", bufs=NCG + 1) as rpool,
            tc.tile_pool(name="scratch", bufs=2) as scratch,
            tc.tile_pool(name="psum", bufs=2, space="PSUM") as psum,
        ):
            gmax_sb = persist.tile([P, GC], f32)
            sexp_sb = persist.tile([P, GC], f32)
            nc.vector.memset(gmax_sb[:], 0.0)
            nc.scalar.memzero(sexp_sb[:])

            # k layout: partition p holds k = 4p + r, r in 0..3; identical
            # permutation on both operands so the contraction is unchanged.
            lview = lhsT_d.rearrange("(p r) j -> p r j", p=P)
            rview = rhsT_d.rearrange("(p r) j -> p r j", p=P)

            def load_l():
                # 1 MB on the Pool SWDGE queue, one 8 KB run per partition
                lt = lpool.tile([P, KT, MR], fp8, tag="lt", name="lt")
                nc.gpsimd.dma_start(lt[:], lview)
                return lt

            def load_r(cg):
                # 1 MB split across both HWDGE queues, 2 KB runs
                rt = rpool.tile([P, KT, CW], fp8, tag="rcb", name=f"rcb{cg}")
                cs = slice(cg * CW, (cg + 1) * CW)
                nc.sync.dma_start(rt[:, 0:2, :], rview[:, 0:2, cs])
                nc.scalar.dma_start(rt[:, 2:4, :], rview[:, 2:4, cs])
                return rt

            def do_group(lt, rt, m, cg, drain):
                pt = psum.tile([P, CW], f32, tag="acc", name=f"acc{m}_{cg}")
                for t in range(KP):
                    for f in range(CW // NFREE):
                        nc.tensor.matmul(
                            pt[:, f * NFREE : (f + 1) * NFREE],
                            lt[:, 2 * t : 2 * t + 2, m * P : (m + 1) * P],
                            rt[:, 2 * t : 2 * t + 2, f * NFREE : (f + 1) * NFREE],
                            start=(t == 0),
                            stop=(t == KP - 1),
                            perf_mode=DR,
                        )
                col = m * NCG + cg
                if drain == "token":
                    nc.vector.tensor_reduce(
                        gmax_sb[:, 0:1], pt[:, 0:2], axis=X, op=Alu.max
                    )
                else:
                    # split the drain across both engines so each tile is
                    # freed in ~1.4us (< the 2.2us group matmul time) and
                    # the next-next group's matmuls never wait on PSUM:
                    # DVE takes the exact max of the first half, ACT the
                    # lse bound of the second half.
                    nc.vector.tensor_reduce(
                        gmax_sb[:, col : col + 1],
                        pt[:, 0 : CW // 2],
                        axis=X,
                        op=Alu.max,
                    )
                    esc = scratch.tile(
                        [P, CW // 2], f32, tag="esc", name=f"esc{m}_{cg}"
                    )
                    nc.scalar.activation(
                        esc[:],
                        pt[:, CW // 2 :],
                        mybir.ActivationFunctionType.Exp,
                        bias=0.0,
                        scale=T_LSE,
                        accum_out=sexp_sb[:, col : col + 1],
                    )

            body_ctx = tc.For_i(0, reps, 1) if reps > 1 else contextlib.nullcontext()

            if mode == "dma":
                with body_ctx:
                    load_l()
                    for cg in range(NCG):
                        load_r(cg)
                    nc.vector.memset(gmax_sb[:, 0:1], 0.0)
                    nc.sync.dma_start(gmax_d[:, 0:1], gmax_sb[:, 0:1])
            elif mode in ("mm", "compute"):
                lt = load_l()
                rts = [load_r(cg) for cg in range(NCG)]
                drain = "token" if mode == "mm" else "real"
                with body_ctx:
                    for cg in range(NCG):
                        for m in range(MT):
                            do_group(lt, rts[cg], m, cg, drain)
                    nc.sync.dma_start(gmax_d[:], gmax_sb[:])
                    nc.scalar.dma_start(sexp_d[:], sexp_sb[:])
            else:
                with body_ctx:
                    lt = load_l()
                    rts = [load_r(cg) for cg in range(NCG)]
                    for cg in range(NCG):
                        for m in range(MT):
                            do_group(lt, rts[cg], m, cg, "real")
                    nc.sync.dma_start(gmax_d[:], gmax_sb[:])
                    nc.scalar.dma_start(sexp_d[:], sexp_sb[:])

    nc.compile()
    return nc


def _get_program():
    key = (1, "full")
    if key not in _PROGS:
        _PROGS[key] = _build_program()
    return _PROGS[key]


def _make_in_maps(o1, o2):
    """Per-core input dicts from full fp32 arrays (clipped to fp8 range)."""
    fp8 = ml_dtypes.float8_e4m3  # TRN E4M3: max normal +-240
    o1c = np.clip(o1, -224.0, 224.0)
    o2c = np.clip(o2, -224.0, 224.0)
    o1hT = np.ascontiguousarray(o1c[:, :HK].astype(fp8).T)  # [HK, N]
    o2hT = np.ascontiguousarray(o2c[:, :HK].astype(fp8).T)  # [HK, N]
    in_maps = []
    for c in range(NCORES):
        rsl = slice((c // CBn) * MR, (c // CBn + 1) * MR)
        csl = slice((c % CBn) * MC, (c % CBn + 1) * MC)
        in_maps.append(
            {
                "lhsT": np.ascontiguousarray(o1hT[:, rsl]),
                "rhsT": np.ascontiguousarray(o2hT[:, csl]),
            }
        )
    return in_maps


def _exact_fallback(o1, o2, rn):
    """Faithful numpy mirror of the reference (fp32 ops, lax.top_k ties)."""
    o1 = o1.astype(np.float32)
    o2 = o2.astype(np.float32)
    pos = ((o2 - o1) ** 2).sum(axis=1, dtype=np.float32)
    a2 = (o1**2).sum(axis=1, dtype=np.float32)
    b2 = (o2**2).sum(axis=1, dtype=np.float32)
    neg = np.empty(N, np.float32)
    rows = np.arange(N)
    blk = 512
    for s in range(0, N, blk):
        e = min(s + blk, N)
        gram = o1[s:e] @ o2.T
        sq = a2[s:e, None] + b2[None, :] - 2.0 * gram
        dist = np.sqrt(np.maximum(sq, 0.0)).astype(np.float32)
        for r in range(s, e):
            drow = dist[r - s]
            part = np.argpartition(drow, QUANT - 1)[: QUANT + 32]
            order = part[np.lexsort((part, drow[part]))]
            v_k = drow[order[QUANT - 1]]
            if (drow == v_k).sum() > (drow[order[:QUANT]] == v_k).sum():
                order = np.lexsort((rows, drow))
            idx = order[:QUANT]
            vals = drow[idx]
            r_sel = int(rn[r]) % QUANT
            if idx[r_sel] == r:
                r_sel = (r_sel + 1) % QUANT
            neg[r] = vals[r_sel]
    neg_loss = np.maximum(np.float32(MARGIN) - neg, np.float32(0.0))
    return np.float32(
        np.mean(pos, dtype=np.float64) + np.mean(neg_loss, dtype=np.float64)
    )


def kernel(output1, output2, rn):
    global LAST_RESULTS, LAST_BOUND, LAST_FASTPATH
    o1 = np.ascontiguousarray(np.asarray(output1, dtype=np.float32))
    o2 = np.ascontiguousarray(np.asarray(output2, dtype=np.float32))
    rn_np = np.asarray(rn)

    in_maps = _make_in_maps(o1, o2)

    from concourse.bass_utils import run_bass_kernel_spmd

    nc = _get_program()
    res = run_bass_kernel_spmd(nc, in_maps, list(range(NCORES)))
    LAST_RESULTS = res

    # Per-row upper bound on the HEAD gram max. Each block column holds
    # the exact max of its first half (gmax, DVE) and an lse upper bound
    # of its second half (sexp, ACT); the block bound is their max.
    ub_head = np.full(N, -np.inf)
    for c in range(NCORES):
        r0 = (c // CBn) * MR
        gm = np.asarray(res.results[c]["gmax"], dtype=np.float64)  # [P, GC]
        sx = np.asarray(res.results[c]["sexp"], dtype=np.float64)  # [P, GC]
        for m in range(MT):
            base = r0 + m * P
            for cg in range(NCG):
                col = m * NCG + cg
                lse = np.log(np.maximum(sx[:, col], 1e-30)) / T_LSE + 1.0
                blk = np.maximum(gm[:, col], lse)
                np.maximum(
                    ub_head[base : base + P], blk, out=ub_head[base : base + P]
                )

    # Rigorous zero-check for the margin term (all fp64 host math):
    # dist^2(i,j) >= a2_i + b2min - 2*(head_ub_i + slack_i + tail_i)
    o1_64 = o1.astype(np.float64)
    o2_64 = o2.astype(np.float64)
    a2 = (o1_64**2).sum(axis=1)
    b2 = (o2_64**2).sum(axis=1)
    ah = np.sqrt((o1_64[:, :HK] ** 2).sum(axis=1))
    bh_max = float(np.sqrt((o2_64[:, :HK] ** 2).sum(axis=1).max()))
    at = np.sqrt((o1_64[:, HK:] ** 2).sum(axis=1))
    bt_max = float(np.sqrt((o2_64[:, HK:] ** 2).sum(axis=1).max()))
    amax = float(np.sqrt(a2.max()))
    bmax = float(np.sqrt(b2.max()))
    # fp8 e4m3 round-to-nearest rel err 2^-4 per matmul input element:
    # |g_fp8 - g_head| <= (2*2^-4 + 2^-8)*|a_h||b_h|, plus fp32 accum noise
    slack = 0.1330 * ah * bh_max + 0.1
    tail = at * bt_max
    clipped = (np.abs(o1) > 224.0).any() or (np.abs(o2) > 224.0).any()
    # the reference computes sq in fp32; cover its roundoff too
    eps_ref = 1e-3 * amax * bmax + 1e-2
    bound = a2 + b2.min() - 2.0 * (ub_head + slack + tail)
    LAST_BOUND = float(bound.min())
    LAST_FASTPATH = (
        not clipped
        and bool(np.isfinite(bound).all())
        and LAST_BOUND >= MARGIN * MARGIN + eps_ref
    )
    if LAST_FASTPATH:
        pos = ((o2_64 - o1_64) ** 2).sum(axis=1)
        return np.float32(np.mean(pos))
    return _exact_fallback(o1, o2, rn_np)


# revision 8
# speedup vs baseline: 1.1294x; 1.1294x over previous
"""ContrastiveLoss kernel for 8x Trainium2 NeuronCores.

Math: loss = mean_i ||o2_i - o1_i||^2  +  mean_i relu(MARGIN - d_i)
where d_i is the rn_i-th smallest entry of row i of the [N,N] distance
matrix dist(i,j) = ||o2_j - o1_i||.

Every candidate d_i is >= min_j dist(i,j), so whenever we can PROVE
min_j dist(i,j) >= MARGIN for all i the margin term is exactly 0 and
loss == mean(pos), which the host computes exactly in fp64.

The proof splits the feature dim into a 512-dim head and 512-dim tail:
    <a_i, b_j> <= <a_head_i, b_head_j> + |a_tail_i| * max_j |b_tail_j|
The head inner products are bounded on-device: each core computes its
fp8 DoubleRow Gram block (the dominant compute) and reduces each
[128, 2048] PSUM tile to either an exact row max (DVE) or a
log-sum-exp upper bound (ACT, sum_j exp(T*g)); the host combines them
with rigorous fp8-rounding slack. The tail norms, full row norms, and
positive loss are tiny O(N*D) host work in fp64. For the reference's
gaussian inputs the certified bound clears the threshold by >200 (see
LAST_BOUND); if the check ever failed, an exact host fallback
reproduces the reference computation.

Sharding: 4 row-blocks x 2 col-halves. Core c owns rows
(c//2)*2048 .. +2048 of output1 and cols (c%2)*4096 .. +4096 of
output2: per-core DMA is 1 MB lhsT + 2 MB rhsT (fp8 head only),
split across the gpsimd SWDGE queue and both HWDGE queues (SP, ACT).
"""

import numpy as np
import ml_dtypes

N = 8192
D = 1024
HK = 512            # head dims bounded on-device; D-HK tail bounded on host
NCORES = 8
RB = 4              # row blocks
CBn = 2             # col halves
MR = N // RB        # 2048 rows per core
MC = N // CBn       # 4096 cols per core
P = 128
KT = HK // P        # 4 k-tiles
KP = KT // 2        # 2 DoubleRow k-pairs
MT = MR // P        # 16 m-tiles per core
NFREE = 512         # matmul free dim
CW = 2048           # psum tile width (4 banks)
NCG = MC // CW      # 2 column groups per core
GC = MT * NCG       # 32 reduce columns per core
MARGIN = 2.0
QUANT = 30
T_LSE = 0.25        # lse temperature: ln(sexp)/T + 1 >= block row max

_PROGS = {}
LAST_RESULTS = None
LAST_BOUND = None
LAST_FASTPATH = None


def _build_program(reps=1, mode="full"):
    """mode: 'full' (normal), 'dma' (loads only), 'mm' (loads hoisted,
    matmuls + token drains), 'compute' (loads hoisted, real drains)."""
    import contextlib

    import concourse.bacc as bacc
    import concourse.mybir as mybir
    import concourse.tile as tile

    nc = bacc.Bacc(None, target_bir_lowering=False, debug=False)
    f32 = mybir.dt.float32
    fp8 = mybir.dt.float8e4
    X = mybir.AxisListType.X
    Alu = mybir.AluOpType
    DR = mybir.MatmulPerfMode.DoubleRow

    lhsT_d = nc.dram_tensor("lhsT", [HK, MR], fp8, kind="ExternalInput")
    rhsT_d = nc.dram_tensor("rhsT", [HK, MC], fp8, kind="ExternalInput")
    # col = m*NCG + cg; (m+cg) even -> DVE exact max in gmax, odd -> ACT
    # sum_j exp(T_LSE*g) in sexp. Unwritten cols of the other tensor stay 0.
    gmax_d = nc.dram_tensor("gmax", [P, GC], f32, kind="ExternalOutput")
    sexp_d = nc.dram_tensor("sexp", [P, GC], f32, kind="ExternalOutput")

    with tile.TileContext(nc) as tc:
        with (
            tc.tile_pool(name="persist", bufs=1) as persist,
            tc.tile_pool(name="lpool", bufs=2) as lpool,
            tc.tile_pool(name="rpool", bufs=NCG + 1) as rpool,
            tc.tile_pool(name="scratch", bufs=2) as scratch,
            tc.tile_pool(name="psum", bufs=2, space="PSUM") as psum,
        ):
            gmax_sb = persist.tile([P, GC], f32)
            sexp_sb = persist.tile([P, GC], f32)
            nc.vector.memset(gmax_sb[:], 0.0)
            nc.scalar.memzero(sexp_sb[:])

            # k layout: partition p holds k = 4p + r, r in 0..3; identical
            # permutation on both operands so the contraction is unchanged.
            lview = lhsT_d.rearrange("(p r) j -> p r j", p=P)
            rview = rhsT_d.rearrange("(p r) j -> p r j", p=P)

            def load_l():
                # 1 MB on the Pool SWDGE queue, one 8 KB run per partition
                lt = lpool.tile([P, KT, MR], fp8, tag="lt", name="lt")
                nc.gpsimd.dma_start(lt[:], lview)
                return lt

            def load_r(cg):
                # 1 MB split across both HWDGE queues, 2 KB runs
                rt = rpool.tile([P, KT, CW], fp8, tag="rcb", name=f"rcb{cg}")
                cs = slice(cg * CW, (cg + 1) * CW)
                nc.sync.dma_start(rt[:, 0:2, :], rview[:, 0:2, cs])
                nc.scalar.dma_start(rt[:, 2:4, :], rview[:, 2:4, cs])
                return rt

            def do_group(lt, rt, m, cg, drain):
                pt = psum.tile([P, CW], f32, tag="acc", name=f"acc{m}_{cg}")
                for t in range(KP):
                    for f in range(CW // NFREE):
                        nc.tensor.matmul(
                            pt[:, f * NFREE : (f + 1) * NFREE],
                            lt[:, 2 * t : 2 * t + 2, m * P : (m + 1) * P],
                            rt[:, 2 * t : 2 * t + 2, f * NFREE : (f + 1) * NFREE],
                            start=(t == 0),
                            stop=(t == KP - 1),
                            perf_mode=DR,
                        )
                col = m * NCG + cg
                if drain == "token":
                    nc.vector.tensor_reduce(
                        gmax_sb[:, 0:1], pt[:, 0:2], axis=X, op=Alu.max
                    )
                elif (m + cg) % 2 == 0:
                    nc.vector.tensor_reduce(
                        gmax_sb[:, col : col + 1], pt[:], axis=X, op=Alu.max
                    )
                else:
                    esc = scratch.tile([P, CW], f32, tag="esc", name=f"esc{m}_{cg}")
                    nc.scalar.activation(
                        esc[:],
                        pt[:],
                        mybir.ActivationFunctionType.Exp,
                        bias=0.0,
                        scale=T_LSE,
                        accum_out=sexp_sb[:, col : col + 1],
                    )

            body_ctx = tc.For_i(0, reps, 1) if reps > 1 else contextlib.nullcontext()

            if mode == "dma":
                with body_ctx:
                    load_l()
                    for cg in range(NCG):
                        load_r(cg)
                    nc.vector.memset(gmax_sb[:, 0:1], 0.0)
                    nc.sync.dma_start(gmax_d[:, 0:1], gmax_sb[:, 0:1])
            elif mode in ("mm", "compute"):
                lt = load_l()
                rts = [load_r(cg) for cg in range(NCG)]
                drain = "token" if mode == "mm" else "real"
                with body_ctx:
                    for cg in range(NCG):
                        for m in range(MT):
                            do_group(lt, rts[cg], m, cg, drain)
                    nc.sync.dma_start(gmax_d[:], gmax_sb[:])
                    nc.scalar.dma_start(sexp_d[:], sexp_sb[:])
            else:
                with body_ctx:
                    lt = load_l()
                    rts = [load_r(cg) for cg in range(NCG)]
                    for cg in range(NCG):
                        for m in range(MT):
                            do_group(lt, rts[cg], m, cg, "real")
                    nc.sync.dma_start(gmax_d[:], gmax_sb[:])
                    nc.scalar.dma_start(sexp_d[:], sexp_sb[:])

    nc.compile()
    return nc


def _get_program():
    key = (1, "full")
    if key not in _PROGS:
        _PROGS[key] = _build_program()
    return _PROGS[key]


def _make_in_maps(o1, o2):
    """Per-core input dicts from full fp32 arrays (clipped to fp8 range)."""
    fp8 = ml_dtypes.float8_e4m3  # TRN E4M3: max normal +-240
    o1c = np.clip(o1, -224.0, 224.0)
    o2c = np.clip(o2, -224.0, 224.0)
    o1hT = np.ascontiguousarray(o1c[:, :HK].astype(fp8).T)  # [HK, N]
    o2hT = np.ascontiguousarray(o2c[:, :HK].astype(fp8).T)  # [HK, N]
    in_maps = []
    for c in range(NCORES):
        rsl = slice((c // CBn) * MR, (c // CBn + 1) * MR)
        csl = slice((c % CBn) * MC, (c % CBn + 1) * MC)
        in_maps.append(
            {
                "lhsT": np.ascontiguousarray(o1hT[:, rsl]),
                "rhsT": np.ascontiguousarray(o2hT[:, csl]),
            }
        )
    return in_maps


def _exact_fallback(o1, o2, rn):
    """Faithful numpy mirror of the reference (fp32 ops, lax.top_k ties)."""
    o1 = o1.astype(np.float32)
    o2 = o2.astype(np.float32)
    pos = ((o2 - o1) ** 2).sum(axis=1, dtype=np.float32)
    a2 = (o1**2).sum(axis=1, dtype=np.float32)
    b2 = (o2**2).sum(axis=1, dtype=np.float32)
    neg = np.empty(N, np.float32)
    rows = np.arange(N)
    blk = 512
    for s in range(0, N, blk):
        e = min(s + blk, N)
        gram = o1[s:e] @ o2.T
        sq = a2[s:e, None] + b2[None, :] - 2.0 * gram
        dist = np.sqrt(np.maximum(sq, 0.0)).astype(np.float32)
        for r in range(s, e):
            drow = dist[r - s]
            part = np.argpartition(drow, QUANT - 1)[: QUANT + 32]
            order = part[np.lexsort((part, drow[part]))]
            v_k = drow[order[QUANT - 1]]
            if (drow == v_k).sum() > (drow[order[:QUANT]] == v_k).sum():
                order = np.lexsort((rows, drow))
            idx = order[:QUANT]
            vals = drow[idx]
            r_sel = int(rn[r]) % QUANT
            if idx[r_sel] == r:
                r_sel = (r_sel + 1) % QUANT
            neg[r] = vals[r_sel]
    neg_loss = np.maximum(np.float32(MARGIN) - neg, np.float32(0.0))
    return np.float32(
        np.mean(pos, dtype=np.float64) + np.mean(neg_loss, dtype=np.float64)
    )


def kernel(output1, output2, rn):
    global LAST_RESULTS, LAST_BOUND, LAST_FASTPATH
    o1 = np.ascontiguousarray(np.asarray(output1, dtype=np.float32))
    o2 = np.ascontiguousarray(np.asarray(output2, dtype=np.float32))
    rn_np = np.asarray(rn)

    in_maps = _make_in_maps(o1, o2)

    from concourse.bass_utils import run_bass_kernel_spmd

    nc = _get_program()
    res = run_bass_kernel_spmd(nc, in_maps, list(range(NCORES)))
    LAST_RESULTS = res

    # Per-row upper bound on the HEAD gram max, combining the exact-max
    # (even) and lse (odd) block columns from both col-half cores.
    ub_head = np.full(N, -np.inf)
    for c in range(NCORES):
        r0 = (c // CBn) * MR
        gm = np.asarray(res.results[c]["gmax"], dtype=np.float64)  # [P, GC]
        sx = np.asarray(res.results[c]["sexp"], dtype=np.float64)  # [P, GC]
        for m in range(MT):
            base = r0 + m * P
            for cg in range(NCG):
                col = m * NCG + cg
                if (m + cg) % 2 == 0:
                    blk = gm[:, col]
                else:
                    blk = np.log(np.maximum(sx[:, col], 1e-30)) / T_LSE + 1.0
                np.maximum(
                    ub_head[base : base + P], blk, out=ub_head[base : base + P]
                )

    # Rigorous zero-check for the margin term (all fp64 host math):
    # dist^2(i,j) >= a2_i + b2min - 2*(head_ub_i + slack_i + tail_i)
    o1_64 = o1.astype(np.float64)
    o2_64 = o2.astype(np.float64)
    a2 = (o1_64**2).sum(axis=1)
    b2 = (o2_64**2).sum(axis=1)
    ah = np.sqrt((o1_64[:, :HK] ** 2).sum(axis=1))
    bh_max = float(np.sqrt((o2_64[:, :HK] ** 2).sum(axis=1).max()))
    at = np.sqrt((o1_64[:, HK:] ** 2).sum(axis=1))
    bt_max = float(np.sqrt((o2_64[:, HK:] ** 2).sum(axis=1).max()))
    amax = float(np.sqrt(a2.max()))
    bmax = float(np.sqrt(b2.max()))
    # fp8 e4m3 round-to-nearest rel err 2^-4 per matmul input element:
    # |g_fp8 - g_head| <= (2*2^-4 + 2^-8)*|a_h||b_h|, plus fp32 accum noise
    slack = 0.1330 * ah * bh_max + 0.1
    tail = at * bt_max
    clipped = (np.abs(o1) > 224.0).any() or (np.abs(o2) > 224.0).any()
    # the reference computes sq in fp32; cover its roundoff too
    eps_ref = 1e-3 * amax * bmax + 1e-2
    bound = a2 + b2.min() - 2.0 * (ub_head + slack + tail)
    LAST_BOUND = float(bound.min())
    LAST_FASTPATH = (
        not clipped
        and bool(np.isfinite(bound).all())
        and LAST_BOUND >= MARGIN * MARGIN + eps_ref
    )
    if LAST_FASTPATH:
        pos = ((o2_64 - o1_64) ** 2).sum(axis=1)
        return np.float32(np.mean(pos))
    return _exact_fallback(o1, o2, rn_np)
